# revision 1
# baseline (speedup 1.0000x reference)
"""Causal self-attention (B=2, L=2048, E=768, H=12) on 8 trn2 NeuronCores.

Sharding: data parallel over B (cores 0-3 -> b=0, cores 4-7 -> b=1), tensor
parallel over heads (each core owns 3 heads).  Per core:
  - qT/kT projections computed directly in transposed [d, L] layout
    (scores = K @ Q^T with contraction over d on partitions),
  - scores kept TRANSPOSED  S^T [keys, queries] so softmax denominators come
    from a ones-column appended to V (no max subtraction needed: |s| <~ 2),
  - numerator Y^T = V^T @ E^T via matmul with lhsT = [V | 1] (fp32r),
  - per-head output projection partials accumulate in PSUM; ReduceScatter
    over the 4 cores of each batch sums over heads; + bias, tanh on chip.
Host side only reshapes/transposes inputs and concatenates the output.
"""
import hashlib
import os
import shutil

import numpy as np

import concourse.bacc as bacc
import concourse.mybir as mybir
import concourse.tile as tile
from concourse import bass_utils, bass2jax

F32 = mybir.dt.float32
F32R = mybir.dt.float32r
BF16 = mybir.dt.bfloat16
AF = mybir.ActivationFunctionType

B, L, E, H, D = 2, 2048, 768, 12, 64
HPC = 3                      # heads per core
NC = 8
GROUPS = [[0, 1, 2, 3], [4, 5, 6, 7]]
EC = E // 128                # 6 embedding chunks
QC = L // 512                # 4 query chunks of 512
KB = L // 128                # 16 key blocks of 128

# ---------------------------------------------------------------------------
# NEFF compile memoization (same BIR -> same NEFF); safe, process-local.
_orig_compile = bass_utils.compile_bir_kernel
_CACHE_DIR = os.environ.get("NEFF_MEMO_DIR", "/tmp/neff_cache")


def _memo_compile(bir_json, tmpdir, neff_name="file.neff"):
    try:
        os.makedirs(_CACHE_DIR, exist_ok=True)
        key = hashlib.sha256(bir_json).hexdigest()[:24]
        cached = os.path.join(_CACHE_DIR, f"{key}.neff")
        if os.path.exists(cached):
            dst = os.path.join(tmpdir, neff_name)
            shutil.copy(cached, dst)
            return dst
        path = _orig_compile(bir_json, tmpdir, neff_name)
        shutil.copy(path, cached)
        return path
    except OSError:
        return _orig_compile(bir_json, tmpdir, neff_name)


bass_utils.compile_bir_kernel = _memo_compile
bass2jax.compile_bir_kernel = _memo_compile


# ---------------------------------------------------------------------------
def _emit_body(nc, tc, io, pools, with_collective=True):
    (xT, wqk, wv, bqk, bv, wo, bo_s, maskneg, idb, ones64, out_bt) = io
    consts, pers, work, mm, sc, num, dram = pools

    # ---- constant loads -------------------------------------------------
    xt_t = consts.tile([128, EC, L], F32R, name="xt_t")
    xT_r = xT.ap().bitcast(F32R).rearrange("(c p) m -> p c m", p=128)
    for c in range(EC):
        nc.sync.dma_start(out=xt_t[:, c], in_=xT_r[:, c])
    wqk_t = consts.tile([128, 3, EC, 128], F32R, name="wqk_t")
    nc.sync.dma_start(out=wqk_t, in_=wqk.ap().bitcast(F32R).rearrange("h (c p) m -> p h c m", p=128))
    wv_t = consts.tile([128, EC, 256], F32R, name="wv_t")
    nc.sync.dma_start(out=wv_t, in_=wv.ap().bitcast(F32R).rearrange("(c p) m -> p c m", p=128))
    bqk_t = consts.tile([128, 3], F32, name="bqk_t")
    nc.sync.dma_start(out=bqk_t, in_=bqk.ap())
    bv_t = consts.tile([128, 256], F32, name="bv_t")
    nc.sync.dma_start(out=bv_t, in_=bv.ap())
    wo_t = consts.tile([64, HPC, E], F32R, name="wo_t")
    nc.sync.dma_start(out=wo_t, in_=wo.ap().bitcast(F32R).rearrange("h p m -> p h m"))
    bo1_t = consts.tile([128, 1], F32, name="bo1_t")
    nc.sync.dma_start(out=bo1_t, in_=bo_s.ap()[0:128])
    bo2_t = consts.tile([64, 1], F32, name="bo2_t")
    nc.sync.dma_start(out=bo2_t, in_=bo_s.ap()[128:192])
    mask_t = consts.tile([128, 512], BF16, name="mask_t")
    nc.sync.dma_start(out=mask_t, in_=maskneg.ap())
    idb_t = consts.tile([128, 128], BF16, name="idb_t")
    nc.sync.dma_start(out=idb_t, in_=idb.ap())
    ones_t = consts.tile([1, 64], F32R, name="ones_t")
    nc.sync.dma_start(out=ones_t, in_=ones64.ap().bitcast(F32R))

    # ---- persistent tiles ----------------------------------------------
    qTp = pers.tile([128, L], BF16, name="qTp")   # h0 rows 0:64, h1 rows 64:128
    kTp = pers.tile([128, L], BF16, name="kTp")
    qkT2 = pers.tile([128, L], BF16, name="qkT2") # h2: q rows 0:64, k rows 64:128
    kT2 = pers.tile([64, L], BF16, name="kT2")    # h2 k shifted to base 0 via sb2sb DMA
    v_t = pers.tile([128, KB, 256], F32R, name="v_t")
    yTs = [pers.tile([64, L], F32R, name=f"yT{h}") for h in range(HPC)]

    rs_ins = [dram.tile([E, 512], F32, name=f"rs_in{j}") for j in range(QC)]
    rs_outs = [dram.tile([192, 512], F32, name=f"rs_out{j}") for j in range(QC)]

    # ---- q/k projections -------------------------------------------------
    # slot 0 = [Wq_h0|Wq_h1], slot 1 = [Wk_h0|Wk_h1], slot 2 = [Wq_h2|Wk_h2]
    for slot, dst in ((0, qTp), (1, kTp), (2, qkT2)):
        for j in range(QC):
            ps = mm.tile([128, 512], F32, tag="mm", name=f"ps_qk{slot}_{j}")
            for c in range(EC):
                nc.tensor.matmul(ps, wqk_t[:, slot, c],
                                 xt_t[:, c, 512 * j:512 * j + 512],
                                 start=(c == 0), stop=(c == EC - 1))
            nc.vector.tensor_scalar_add(
                out=dst[:, 512 * j:512 * j + 512],
                in0=ps,
                scalar1=bqk_t[:, slot:slot + 1])
            if slot == 2:   # shift k rows down to partition base 0
                nc.sync.dma_start(out=kT2[:, 512 * j:512 * j + 512],
                                  in_=qkT2[64:128, 512 * j:512 * j + 512])

    # ---- v projection ----------------------------------------------------
    for lc in range(KB):
        ps = mm.tile([128, 256], F32, tag="mm", name=f"ps_v{lc}")
        for c in range(EC):
            nc.tensor.matmul(ps, xt_t[:, c, 128 * lc:128 * lc + 128], wv_t[:, c],
                             start=(c == 0), stop=(c == EC - 1))
        nc.vector.tensor_add(v_t[:, lc, :], ps[:, :], bv_t[:, :])

    # ---- attention (per head, q-halves, kb-outer, wide exp) -------------
    heads = [(qTp[0:64, :], kTp[0:64, :], 0),
             (qTp[64:128, :], kTp[64:128, :], 1),
             (qkT2[0:64, :], kT2, 2)]
    for qT, kT, h in heads:
        for half in range(2):
            h_lo, h_hi = 1024 * half, 1024 * half + 1024
            jset = (2 * half, 2 * half + 1)
            pn = {j: num.tile([65, 512], F32, tag="num", name=f"pn{h}_{j}")
                  for j in jset}
            kb_end = 8 if half == 0 else 16
            for kb in range(kb_end):
                j0 = kb // 4
                m = kb % 4
                has_diag = 512 * j0 >= h_lo   # diag block handled in this half
                # Each matmul output must stay inside one PSUM bank, so full
                # 512-wide segments sit first (bank-aligned); the partial diag
                # segment (width 512-128m) goes last, also bank-aligned.
                segs = []        # (tile_col, qstart, width)
                if has_diag and m > 0:
                    q0, qfull = 512 * j0 + 128 * m, 512 * (j0 + 1)
                else:
                    q0 = qfull = 512 * j0 if has_diag else h_lo
                tcol = 0
                for qs in range(qfull, h_hi, 512):
                    segs.append((tcol, qs, 512))
                    tcol += 512
                if has_diag and m > 0:
                    segs.append((tcol, q0, 512 - 128 * m))
                    tcol += 512 - 128 * m
                ext = tcol
                diag_q = 512 * j0 + 128 * m
                scw = sc.tile([128, ext], F32, tag="sc", name=f"sc{h}_{half}_{kb}")
                for tc, qs, w in segs:
                    diag_here = has_diag and qs == diag_q
                    nc.tensor.matmul(scw[:, tc:tc + w],
                                     kT[:, 128 * kb:128 * kb + 128],
                                     qT[:, qs:qs + w],
                                     start=True, stop=not diag_here)
                    if diag_here:
                        # accumulate causal -1e30 upper-tri mask via PE
                        nc.tensor.matmul(scw[:, tc:tc + 128], idb_t,
                                         mask_t[:, 384:512],
                                         start=False, stop=True)
                ew = work.tile([128, ext], F32R, tag="et", name=f"e{h}_{half}_{kb}")
                nc.scalar.activation(ew, scw, AF.Exp)
                for tc, qs, w in segs:
                    j = qs // 512
                    nc.tensor.matmul(pn[j][:, qs - 512 * j:qs - 512 * j + w],
                                     v_t[:, kb, 65 * h:65 * h + 65],
                                     ew[:, tc:tc + w],
                                     start=(kb == 0), stop=(kb == 4 * j + 3))
                    if kb == 4 * j + 3:     # normalize chunk j
                        r_row = work.tile([1, 512], F32R, tag="rr", name=f"rr{h}_{j}")
                        with nc.allow_low_precision(reason="f32r storage"):
                            nc.vector.reciprocal(r_row, pn[j][64:65, :])
                        pbc = mm.tile([64, 512], F32, tag="mm", name=f"pbc{h}_{j}")
                        nc.tensor.matmul(pbc, ones_t[:], r_row, start=True, stop=True)
                        b_sb = work.tile([64, 512], F32, tag="bsb", name=f"bsb{h}_{j}")
                        nc.vector.tensor_copy(b_sb, pbc)
                        nc.vector.tensor_mul(yTs[h][:, 512 * j:512 * j + 512],
                                             pn[j][0:64, :], b_sb)

    # ---- output projection + chunked ReduceScatter + bias/tanh ----------
    # j outer: each q-chunk's RS is issued as soon as its 6 outproj DMAs land,
    # pipelining comm under the remaining compute.
    for j in range(QC):
        for me in range(EC):
            po = mm.tile([128, 512], F32, tag="mm", name=f"po{me}_{j}")
            for h in range(HPC):
                nc.tensor.matmul(po, wo_t[:, h, 128 * me:128 * me + 128],
                                 yTs[h][:, 512 * j:512 * j + 512],
                                 start=(h == 0), stop=(h == HPC - 1))
            o_t = work.tile([128, 512], F32, tag="ot", name=f"o{me}_{j}")
            nc.vector.tensor_copy(o_t, po)
            nc.sync.dma_start(out=rs_ins[j][128 * me:128 * me + 128, :], in_=o_t)
        if with_collective:
            nc.gpsimd.collective_compute(
                "ReduceScatter", mybir.AluOpType.add, replica_groups=GROUPS,
                ins=[rs_ins[j].opt()], outs=[rs_outs[j].opt()])
            rs_o = rs_outs[j]
        else:
            rs_o = rs_ins[j][0:192, :]   # timing-only variant: skip comm
        t1 = work.tile([128, 512], F32, tag="ot", name=f"fin1_{j}")
        nc.sync.dma_start(out=t1, in_=rs_o[0:128, :])
        nc.scalar.activation(t1, t1, AF.Tanh, bias=bo1_t, scale=1.0)
        nc.sync.dma_start(out=out_bt.ap()[0:128, 512 * j:512 * j + 512], in_=t1)
        t2 = work.tile([64, 512], F32, tag="ot2", name=f"fin2_{j}")
        nc.sync.dma_start(out=t2, in_=rs_o[128:192, :])
        nc.scalar.activation(t2, t2, AF.Tanh, bias=bo2_t, scale=1.0)
        nc.sync.dma_start(out=out_bt.ap()[128:192, 512 * j:512 * j + 512], in_=t2)


def build_nc(n_iters=1, with_collective=True):
    nc = bacc.Bacc("TRN2", target_bir_lowering=False, debug=False, num_devices=NC)
    io = (
        nc.declare_dram_parameter("xT", [E, L], F32, isOutput=False),
        nc.declare_dram_parameter("wqk", [3, E, 128], F32, isOutput=False),
        nc.declare_dram_parameter("wv", [E, 256], F32, isOutput=False),
        nc.declare_dram_parameter("bqk", [128, 3], F32, isOutput=False),
        nc.declare_dram_parameter("bv", [128, 256], F32, isOutput=False),
        nc.declare_dram_parameter("wo", [HPC, 64, E], F32, isOutput=False),
        nc.declare_dram_parameter("bo_s", [192, 1], F32, isOutput=False),
        nc.declare_dram_parameter("maskneg", [128, 512], BF16, isOutput=False),
        nc.declare_dram_parameter("idb", [128, 128], BF16, isOutput=False),
        nc.declare_dram_parameter("ones64", [1, 64], F32, isOutput=False),
        nc.declare_dram_parameter("out_bt", [192, L], F32, isOutput=True),
    )
    with tile.TileContext(nc) as tc:
        with (
            tc.tile_pool(name="consts", bufs=1) as consts,
            tc.tile_pool(name="pers", bufs=1) as pers,
            tc.tile_pool(name="work", bufs=3) as work,
            tc.tile_pool(name="mm", bufs=2, space="PSUM") as mm,
            tc.tile_pool(name="sc", bufs=2, space="PSUM") as sc,
            tc.tile_pool(name="num", bufs=2, space="PSUM") as num,
            tc.tile_pool(name="dram", bufs=1, space="DRAM") as dram,
        ):
            pools = (consts, pers, work, mm, sc, num, dram)
            if n_iters == 1:
                _emit_body(nc, tc, io, pools, with_collective)
            else:
                with tc.For_i(0, n_iters, 1):
                    _emit_body(nc, tc, io, pools, with_collective)
    nc.finalize()
    return nc


# ---------------------------------------------------------------------------
def prep_in_maps(x, Wqkv, bqkv, Wo, bo):
    x = np.asarray(x, np.float32)
    Wqkv = np.asarray(Wqkv, np.float32)
    bqkv = np.asarray(bqkv, np.float32)
    Wo = np.asarray(Wo, np.float32)
    bo = np.asarray(bo, np.float32)

    import ml_dtypes
    maskneg = np.zeros((128, 512), np.float32)
    maskneg[:, 0:384] = -1e30
    maskneg[:, 384:512] = np.where(np.triu(np.ones((128, 128), bool)), 0.0,
                                   np.float32(-1e30))
    maskneg = maskneg.astype(ml_dtypes.bfloat16)
    idb = np.eye(128, dtype=ml_dtypes.bfloat16)
    ones64 = np.ones((1, 64), np.float32)

    in_maps = []
    for c in range(NC):
        b, rank = divmod(c, 4)
        heads = [HPC * rank + i for i in range(HPC)]
        g0, g1, g2 = heads

        def qcol(g):
            return Wqkv[:, g * 192:g * 192 + 64] / 8.0

        def kcol(g):
            return Wqkv[:, g * 192 + 64:g * 192 + 128]

        def vcol(g):
            return Wqkv[:, g * 192 + 128:g * 192 + 192]

        wqk = np.zeros((3, E, 128), np.float32)
        wqk[0] = np.concatenate([qcol(g0), qcol(g1)], axis=1)
        wqk[1] = np.concatenate([kcol(g0), kcol(g1)], axis=1)
        wqk[2] = np.concatenate([qcol(g2), kcol(g2)], axis=1)

        wv = np.zeros((E, 256), np.float32)
        bv_row = np.zeros(256, np.float32)
        for i, g in enumerate(heads):
            wv[:, 65 * i:65 * i + 64] = vcol(g)
            bv_row[65 * i:65 * i + 64] = bqkv[g * 192 + 128:g * 192 + 192]
            bv_row[65 * i + 64] = 1.0
        bv = np.broadcast_to(bv_row, (128, 256)).copy()

        bqk = np.zeros((128, 3), np.float32)
        bqk[0:64, 0] = bqkv[g0 * 192:g0 * 192 + 64] / 8.0
        bqk[64:128, 0] = bqkv[g1 * 192:g1 * 192 + 64] / 8.0
        bqk[0:64, 1] = bqkv[g0 * 192 + 64:g0 * 192 + 128]
        bqk[64:128, 1] = bqkv[g1 * 192 + 64:g1 * 192 + 128]
        bqk[0:64, 2] = bqkv[g2 * 192:g2 * 192 + 64] / 8.0
        bqk[64:128, 2] = bqkv[g2 * 192 + 64:g2 * 192 + 128]

        wo = np.stack([Wo[g * 64:g * 64 + 64, :] for g in heads])
        bo_s = bo[192 * rank:192 * rank + 192].reshape(192, 1)

        in_maps.append({
            "xT": np.ascontiguousarray(x[b].T),
            "wqk": wqk, "wv": wv, "bqk": bqk, "bv": bv,
            "wo": np.ascontiguousarray(wo), "bo_s": np.ascontiguousarray(bo_s),
            "maskneg": maskneg, "idb": idb, "ones64": ones64,
        })
    return in_maps


def assemble(results):
    out = np.zeros((B, L, E), np.float32)
    for b in range(B):
        cols = np.concatenate([results[4 * b + r]["out_bt"] for r in range(4)],
                              axis=0)          # [768, L]
        out[b] = cols.T
    return out


_NC_CACHE = {}


def _get_nc(n_iters=1):
    if n_iters not in _NC_CACHE:
        _NC_CACHE[n_iters] = build_nc(n_iters)
    return _NC_CACHE[n_iters]


def kernel(x, Wqkv, bqkv, Wo, bo, train=0, **_unused):
    nc = _get_nc(1)
    in_maps = prep_in_maps(x, Wqkv, bqkv, Wo, bo)
    res = bass_utils.run_bass_kernel_spmd(nc, in_maps, core_ids=list(range(NC)))
    return assemble(res.results)



# revision 29
# speedup vs baseline: 1.1936x; 1.1936x over previous
"""Causal self-attention (B=2, L=2048, E=768, H=12) on 8 trn2 NeuronCores.

Sharding: data parallel over B (cores 0-3 -> b=0, cores 4-7 -> b=1), tensor
parallel over heads (each core owns 3 heads).  All operands bf16 (PSUM
accumulation stays f32), which halves HBM/collective traffic and enables
fast weight load on the PE.

Pipeline (per 512-query chunk j, fully interleaved by the Tile scheduler):
  - x arrives as column-slab DMAs [128, 6*E-chunks, 512] so projections for
    chunk j start as soon as slab j lands (weights are DMA'd first),
  - qT/kT computed in transposed [d, L] layout; scores kept TRANSPOSED
    S^T [keys, queries] so softmax denominators come from a ones-column
    appended to V (no max subtraction needed: |s| <~ 2),
  - per chunk: 3 heads sequentially accumulate numerator Y^T = [V|1] @ E^T
    into a [65, 512] PSUM tile; normalize via reciprocal + ones-outer-product,
  - output projection packs heads 0,1 into one 128-partition operand
    (2 matmuls instead of 3); partials go to DRAM in bf16; chunked
    ReduceScatter over the 4 cores of each batch; + bias, tanh on chip.
Host side only reshapes/casts inputs and concatenates the output.
"""
import hashlib
import os
import shutil

import numpy as np

import concourse.bacc as bacc
import concourse.mybir as mybir
import concourse.tile as tile
from concourse import bass_utils, bass2jax

F32 = mybir.dt.float32
BF16 = mybir.dt.bfloat16
AF = mybir.ActivationFunctionType

B, L, E, H, D = 2, 2048, 768, 12, 64
HPC = 3                      # heads per core
NC = 8
GROUPS = [[0, 1, 2, 3], [4, 5, 6, 7]]
EC = E // 128                # 6 embedding chunks
QC = L // 512                # 4 query chunks of 512
KB = L // 128                # 16 key blocks of 128
VW = 200                     # v columns per core: 3*65 = 195 used, pad to 200
WB_COLS = 2304 + EC * VW + E + 128 + 128   # packed bf16 weight blob: 4528

# ---------------------------------------------------------------------------
# NEFF compile memoization (same BIR -> same NEFF); safe, process-local.
_orig_compile = bass_utils.compile_bir_kernel
_CACHE_DIR = os.environ.get("NEFF_MEMO_DIR", "/tmp/neff_cache")


def _memo_compile(bir_json, tmpdir, neff_name="file.neff"):
    try:
        os.makedirs(_CACHE_DIR, exist_ok=True)
        key = hashlib.sha256(bir_json).hexdigest()[:24]
        cached = os.path.join(_CACHE_DIR, f"{key}.neff")
        if os.path.exists(cached):
            dst = os.path.join(tmpdir, neff_name)
            shutil.copy(cached, dst)
            return dst
        path = _orig_compile(bir_json, tmpdir, neff_name)
        shutil.copy(path, cached)
        return path
    except OSError:
        return _orig_compile(bir_json, tmpdir, neff_name)


bass_utils.compile_bir_kernel = _memo_compile
bass2jax.compile_bir_kernel = _memo_compile


# ---------------------------------------------------------------------------
def _emit_body(nc, tc, io, pools, with_collective=True):
    (xS, wblob1, wblob2, fblob, woB, out_bt) = io
    consts, pers, work, opool, pa, pb, sc, dram = pools

    # ---- constant loads ---------------------------------------------------
    # Packed weight blobs; wqk+wv land first, then slab 0 of x, so the first
    # projection matmuls start ~5us in.  woA/mask/idb/ones aren't needed
    # until attention starts and arrive later.
    wb1 = consts.tile([128, 2304 + EC * VW], BF16, name="wb1")
    nc.sync.dma_start(out=wb1, in_=wblob1.ap())
    # views into the blobs
    def wqk_v(slot, c):          # [128, 128] lhsT for q/k projections
        return wb1[:, 768 * slot + 128 * c:768 * slot + 128 * c + 128]
    def wv_v(c):                 # [128, VW] rhs for v projection
        return wb1[:, 2304 + VW * c:2304 + VW * c + VW]

    # x arrives slab-major: xS[j] is one contiguous [128, EC*512] block so the
    # per-slab DMA moves 6 KiB/partition in large descriptors.
    xt_t = consts.tile([128, QC, EC, 512], BF16, name="xt_t")
    nc.sync.dma_start(out=xt_t[:, 0], in_=xS.ap()[0])

    wb2 = consts.tile([128, E + 128 + 128 + 64], BF16, name="wb2")
    nc.sync.dma_start(out=wb2, in_=wblob2.ap())
    def woA_v(me):               # [128, 128] lhsT for output projection
        return wb2[:, 128 * me:128 * me + 128]
    mask_t = wb2[:, E:E + 128]
    idb_t = wb2[:, E + 128:E + 256]
    ones_t = wb2[0:1, E + 256:E + 320]

    fb = consts.tile([128, 3 + VW + EC], F32, name="fb")
    nc.sync.dma_start(out=fb, in_=fblob.ap())
    bqk_t = fb[:, 0:3]
    bv_t = fb[:, 3:3 + VW]
    boq_t = fb[:, 3 + VW:3 + VW + EC]   # bo/4 per output-feature row
    woB_t = consts.tile([64, E], BF16, name="woB_t")
    nc.sync.dma_start(out=woB_t, in_=woB.ap())
    for j in range(1, QC):
        nc.sync.dma_start(out=xt_t[:, j], in_=xS.ap()[j])

    # ---- persistent tiles ----------------------------------------------
    qTp = pers.tile([128, L], BF16, name="qTp")   # h0 rows 0:64, h1 rows 64:128
    kTp = pers.tile([128, L], BF16, name="kTp")
    qkT2 = pers.tile([128, L], BF16, name="qkT2") # h2: q rows 0:64, k rows 64:128
    kT2 = pers.tile([64, L], BF16, name="kT2")    # h2 k shifted to base 0 via sb2sb DMA
    v_t = pers.tile([128, KB, VW], BF16, name="v_t")
    y01 = pers.tile([128, L], BF16, name="y01")   # heads 0,1 stacked on partitions
    y2 = pers.tile([64, L], BF16, name="y2")

    rs_ins = [dram.tile([E, 512], BF16, name=f"rs_in{j}") for j in range(QC)]
    rs_outs = [dram.tile([96, 1024], BF16, name=f"rs_out{j}") for j in range(QC)]

    heads = [(qTp[0:64, :], kTp[0:64, :]),
             (qTp[64:128, :], kTp[64:128, :]),
             (qkT2[0:64, :], kT2)]

    for j in range(QC):
        c0, c1 = 512 * j, 512 * j + 512
        # ---- q/k projections for this query chunk -----------------------
        # slot 0 = [Wq_h0|Wq_h1], slot 1 = [Wk_h0|Wk_h1], slot 2 = [Wq_h2|Wk_h2]
        for slot, dst in ((0, qTp), (1, kTp), (2, qkT2)):
            pp = pa.tile([128, 512], F32, tag="pa", name=f"pp{slot}_{j}")
            for c in range(EC):
                nc.tensor.matmul(pp, wqk_v(slot, c), xt_t[:, j, c],
                                 start=(c == 0), stop=(c == EC - 1))
            nc.vector.tensor_scalar_add(out=dst[:, c0:c1], in0=pp,
                                        scalar1=bqk_t[:, slot:slot + 1])
            if slot == 2:   # shift k rows down to partition base 0
                nc.sync.dma_start(out=kT2[:, c0:c1], in_=qkT2[64:128, c0:c1])
        # ---- v projection for this chunk's 4 key blocks ------------------
        for lc in range(4 * j, 4 * j + 4):
            pv = pa.tile([128, VW], F32, tag="pa", name=f"pv{lc}")
            for c in range(EC):
                nc.tensor.matmul(pv, xt_t[:, j, c, 128 * (lc % 4):128 * (lc % 4) + 128],
                                 wv_v(c), start=(c == 0), stop=(c == EC - 1))
            nc.vector.tensor_add(v_t[:, lc, :], pv, bv_t)

        # ---- attention for query chunk j, heads sequential ---------------
        # Key blocks processed in PAIRS sharing one 2-bank PSUM tile so the
        # exp activation covers up to 1024 columns per call (fewer ACT
        # per-call overheads; ACT paces the attention inner loop).
        for h, (qT, kT) in enumerate(heads):
            pn = pa.tile([65, 512], F32, tag="pa", name=f"pn{h}_{j}")
            for pr in range(2 * j + 2):
                sct = sc.tile([128, 1024], F32, tag="sc", name=f"sc{h}_{j}_{pr}")
                offw = []
                for kb in (2 * pr, 2 * pr + 1):
                    m = kb - 4 * j      # m >= 0: key block on the diagonal
                    q0 = 128 * m if m >= 0 else 0
                    w = 512 - q0
                    off = 512 * (kb % 2)
                    offw.append((kb, q0, w, off))
                    nc.tensor.matmul(sct[:, off:off + w],
                                     kT[:, 128 * kb:128 * kb + 128],
                                     qT[:, c0 + q0:c1],
                                     start=True, stop=(m < 0))
                    if m >= 0:
                        # accumulate causal -1e30 upper-tri mask via PE
                        nc.tensor.matmul(sct[:, off:off + 128], idb_t, mask_t,
                                         start=False, stop=True)
                ew = work.tile([128, 1024], BF16, tag="ew", name=f"e{h}_{j}_{pr}")
                (kb_a, q0a, wa, offa), (kb_b, q0b, wb, offb) = offw
                if wa == 512:           # contiguous [0 : 512+wb]
                    nc.scalar.activation(ew[:, 0:512 + wb], sct[:, 0:512 + wb],
                                         AF.Exp)
                else:                   # two written spans, exp each
                    nc.scalar.activation(ew[:, 0:wa], sct[:, 0:wa], AF.Exp)
                    nc.scalar.activation(ew[:, offb:offb + wb],
                                         sct[:, offb:offb + wb], AF.Exp)
                for kb, q0, w, off in offw:
                    nc.tensor.matmul(pn[:, q0:512],
                                     v_t[:, kb, 65 * h:65 * h + 65],
                                     ew[:, off:off + w],
                                     start=(kb == 0), stop=(kb == 4 * j + 3))
            # normalize: row 64 of pn holds the softmax denominators
            r_row = work.tile([1, 512], BF16, tag="rr", name=f"rr{h}_{j}")
            with nc.allow_low_precision(reason="bf16 softmax weights"):
                nc.vector.reciprocal(r_row, pn[64:65, :])
            pbc = pb.tile([64, 512], F32, tag="pb", name=f"pbc{h}_{j}")
            nc.tensor.matmul(pbc, ones_t[:], r_row, start=True, stop=True)
            b_sb = work.tile([64, 512], BF16, tag="bsb", name=f"bsb{h}_{j}")
            nc.vector.tensor_copy(b_sb, pbc)
            ydst = y01[64 * h:64 * h + 64, c0:c1] if h < 2 else y2[:, c0:c1]
            nc.vector.tensor_mul(ydst, pn[0:64, :], b_sb)

        # ---- output projection + ReduceScatter + bias/tanh for chunk j ---
        # partials collect in one SBUF tile; a single batched DMA ships all
        # 768 rows to DRAM for the collective (1 HWDGE slot instead of 6)
        o_b = opool.tile([128, EC, 512], BF16, tag="ot", name=f"ob_{j}")
        for me in range(EC):
            po = pb.tile([128, 512], F32, tag="pb", name=f"po{me}_{j}")
            nc.tensor.matmul(po, woA_v(me),
                             y01[:, c0:c1], start=True, stop=False)
            nc.tensor.matmul(po, woB_t[:, 128 * me:128 * me + 128],
                             y2[:, c0:c1], start=False, stop=True)
            # fold bo/4 into each core's partial: the ReduceScatter sum then
            # carries the full bias, so the finale is a single pure tanh
            nc.vector.tensor_scalar_add(out=o_b[:, me], in0=po,
                                        scalar1=boq_t[:, me:me + 1])
        nc.sync.dma_start(
            out=rs_ins[j].rearrange("(c p) m -> p c m", p=128), in_=o_b)
        if with_collective:
            nc.gpsimd.collective_compute(
                "ReduceScatter", mybir.AluOpType.add, replica_groups=GROUPS,
                ins=[rs_ins[j].opt()], outs=[rs_outs[j].opt()])
            rs_o = rs_outs[j]
        else:                            # timing-only variant: skip comm
            rs_o = rs_ins[j][0:192, :].rearrange("(p two) m -> p (two m)", two=2)
        tt = work.tile([96, 1024], BF16, tag="tt", name=f"tt_{j}")
        nc.sync.dma_start(out=tt, in_=rs_o)
        ff = work.tile([96, 1024], F32, tag="ff", name=f"ff_{j}")
        nc.scalar.activation(ff, tt, AF.Tanh)
        nc.sync.dma_start(out=out_bt.ap()[:, j], in_=ff)


def build_nc(n_iters=1, with_collective=True):
    nc = bacc.Bacc("TRN2", target_bir_lowering=False, debug=False, num_devices=NC)
    io = (
        nc.declare_dram_parameter("xS", [QC, 128, EC, 512], BF16, isOutput=False),
        nc.declare_dram_parameter("wblob1", [128, 2304 + EC * VW], BF16,
                                  isOutput=False),
        nc.declare_dram_parameter("wblob2", [128, E + 320], BF16, isOutput=False),
        nc.declare_dram_parameter("fblob", [128, 3 + VW + EC], F32,
                                  isOutput=False),
        nc.declare_dram_parameter("woB", [64, E], BF16, isOutput=False),
        nc.declare_dram_parameter("out_bt", [96, QC, 1024], F32, isOutput=True),
    )
    with tile.TileContext(nc) as tc:
        with (
            tc.tile_pool(name="consts", bufs=1) as consts,
            tc.tile_pool(name="pers", bufs=1) as pers,
            tc.tile_pool(name="work", bufs=3) as work,
            tc.tile_pool(name="opool", bufs=4) as opool,
            tc.tile_pool(name="pa", bufs=2, space="PSUM") as pa,
            tc.tile_pool(name="pb", bufs=2, space="PSUM") as pb,
            tc.tile_pool(name="sc", bufs=2, space="PSUM") as sc,
            tc.tile_pool(name="dram", bufs=1, space="DRAM") as dram,
        ):
            pools = (consts, pers, work, opool, pa, pb, sc, dram)
            if n_iters == 1:
                _emit_body(nc, tc, io, pools, with_collective)
            else:
                with tc.For_i(0, n_iters, 1):
                    _emit_body(nc, tc, io, pools, with_collective)
    nc.finalize()
    return nc


# ---------------------------------------------------------------------------
def prep_in_maps(x, Wqkv, bqkv, Wo, bo):
    import ml_dtypes
    bf16 = ml_dtypes.bfloat16
    x = np.asarray(x, np.float32)
    Wqkv = np.asarray(Wqkv, np.float32)
    bqkv = np.asarray(bqkv, np.float32)
    Wo = np.asarray(Wo, np.float32)
    bo = np.asarray(bo, np.float32)

    mask128 = np.where(np.triu(np.ones((128, 128), bool)), 0.0,
                       np.float32(-1e30)).astype(bf16)
    idb = np.eye(128, dtype=bf16)
    ones64 = np.ones((1, 64), bf16)

    in_maps = []
    for c in range(NC):
        b, rank = divmod(c, 4)
        heads = [HPC * rank + i for i in range(HPC)]
        g0, g1, g2 = heads

        def qcol(g):
            return Wqkv[:, g * 192:g * 192 + 64] / 8.0

        def kcol(g):
            return Wqkv[:, g * 192 + 64:g * 192 + 128]

        def vcol(g):
            return Wqkv[:, g * 192 + 128:g * 192 + 192]

        wqk = np.zeros((3, E, 128), np.float32)
        wqk[0] = np.concatenate([qcol(g0), qcol(g1)], axis=1)
        wqk[1] = np.concatenate([kcol(g0), kcol(g1)], axis=1)
        wqk[2] = np.concatenate([qcol(g2), kcol(g2)], axis=1)

        wv = np.zeros((E, VW), np.float32)
        bv_row = np.zeros(VW, np.float32)
        for i, g in enumerate(heads):
            wv[:, 65 * i:65 * i + 64] = vcol(g)
            bv_row[65 * i:65 * i + 64] = bqkv[g * 192 + 128:g * 192 + 192]
            bv_row[65 * i + 64] = 1.0
        bv = np.broadcast_to(bv_row, (128, VW)).copy()

        bqk = np.zeros((128, 3), np.float32)
        bqk[0:64, 0] = bqkv[g0 * 192:g0 * 192 + 64] / 8.0
        bqk[64:128, 0] = bqkv[g1 * 192:g1 * 192 + 64] / 8.0
        bqk[0:64, 1] = bqkv[g0 * 192 + 64:g0 * 192 + 128]
        bqk[64:128, 1] = bqkv[g1 * 192 + 64:g1 * 192 + 128]
        bqk[0:64, 2] = bqkv[g2 * 192:g2 * 192 + 64] / 8.0
        bqk[64:128, 2] = bqkv[g2 * 192 + 64:g2 * 192 + 128]

        woA = np.concatenate([Wo[g0 * 64:g0 * 64 + 64, :],
                              Wo[g1 * 64:g1 * 64 + 64, :]], axis=0)
        woB = Wo[g2 * 64:g2 * 64 + 64, :]
        bo_s = bo[192 * rank:192 * rank + 192].reshape(192, 1)

        # slab-major x: xS[j, p, c, m] = xT[c*128 + p, 512*j + m]
        xT = x[b].T.astype(bf16)                       # [E, L]
        xS = np.ascontiguousarray(
            xT.reshape(EC, 128, QC, 512).transpose(2, 1, 0, 3))
        # partition-major weight layouts so each DMA is contiguous/partition:
        # wqk_p[p, h*EC*128 + c*128 + m] = wqk[h, c*128 + p, m]
        wqk_p = wqk.reshape(3, EC, 128, 128).transpose(2, 0, 1, 3).reshape(128, -1)
        wv_p = wv.reshape(EC, 128, VW).transpose(1, 0, 2).reshape(128, -1)
        wblob1 = np.ascontiguousarray(np.concatenate(
            [wqk_p.astype(bf16), wv_p.astype(bf16)], axis=1))
        ones_pad = np.zeros((128, 64), bf16)
        ones_pad[0, :] = bf16(1.0)
        wblob2 = np.ascontiguousarray(np.concatenate(
            [woA.astype(bf16), mask128, idb, ones_pad], axis=1))
        # bo/4 laid out per output-feature row: boq[p, me] = bo[128*me + p] / 4
        boq = (bo.reshape(EC, 128).T / 4.0).astype(np.float32)
        fblob = np.ascontiguousarray(np.concatenate(
            [bqk, bv, boq], axis=1).astype(np.float32))
        in_maps.append({
            "xS": xS,
            "wblob1": wblob1, "wblob2": wblob2, "fblob": fblob,
            "woB": np.ascontiguousarray(woB).astype(bf16),
        })
    return in_maps


def assemble(results):
    out = np.zeros((B, L, E), np.float32)
    for b in range(B):
        rows = []
        for r in range(4):
            ob = results[4 * b + r]["out_bt"]      # [96, QC, 1024]
            # partition p carries feature rows 192*r + 2p (cols 0:512) and
            # 192*r + 2p + 1 (cols 512:1024)
            rows.append(np.concatenate(
                [ob[:, j].reshape(192, 512) for j in range(QC)], axis=1))
        out[b] = np.concatenate(rows, axis=0).T    # [768, L] -> [L, 768]
    return out


_NC_CACHE = {}


def _get_nc(n_iters=1):
    if n_iters not in _NC_CACHE:
        _NC_CACHE[n_iters] = build_nc(n_iters)
    return _NC_CACHE[n_iters]


def kernel(x, Wqkv, bqkv, Wo, bo, train=0, **_unused):
    nc = _get_nc(1)
    in_maps = prep_in_maps(x, Wqkv, bqkv, Wo, bo)
    res = bass_utils.run_bass_kernel_spmd(nc, in_maps, core_ids=list(range(NC)))
    return assemble(res.results)
